# revision 13
# baseline (speedup 1.0000x reference)
"""DLinear forward kernel for Trainium2, sharded over 8 NeuronCores.

Model (per channel c):
    season = AvgPool1d(x_c, k=25, pad=12, count_include_pad=True)
    out_c  = (x_c - season) @ W_res[c].T + season @ W_trend[c].T + b_res[c] + b_trend[c]
           = x_c @ W_res[c].T + season @ (W_trend[c]-W_res[c]).T + bias[c]

Sharding: channels split across cores (zero communication). Per core:
  - x slice pre-transposed to [S, Csh, B] so SBUF tiles are [s-chunk, (c,b)]
  - weights pre-transposed to [Csh, S, P] so matmul rhs tiles DMA contiguously
  - season computed on TensorE as a banded-matrix matmul (A is the avg-pool
    operator, symmetric, built host-side)
  - bias injected into PSUM via a K=1 matmul with a ones row
"""

import sys

sys.path.insert(0, "/opt/trn_rl_repo")

from contextlib import ExitStack

import numpy as np

B, S, C, P = 64, 720, 321, 336
WIN, PAD = 25, 12
N_CORES = 8
CSH = 41                      # padded channels per core (8*41 = 328 >= 321)
SCH, NCH = 120, 6             # S split into 6 chunks of 120 partitions
FREE = CSH * B                # 2624 columns in the (c, b) free dim
NT = 512                      # pooling matmul moving-dim tile
F32R = True                   # float32r matmuls (4-byte, full-rate at N>=256)

_cached = {}


def _build_program():
    import concourse.bacc as bacc
    import concourse.mybir as mybir
    import concourse.tile as tile

    f32 = mybir.dt.float32
    f32r = mybir.dt.float32r

    mf = f32r if F32R else f32  # dtype for all matmul operands

    nc = bacc.Bacc("TRN2", target_bir_lowering=False, debug=False,
                   num_devices=N_CORES)
    xt = nc.dram_tensor("xt", [S, CSH, B], mf, kind="ExternalInput").ap()
    wrt = nc.dram_tensor("wrt", [CSH, S, P], mf, kind="ExternalInput").ap()
    # wdt chunks are padded to 121 rows; row 120 of chunk 5 carries the bias
    wdt = nc.dram_tensor("wdt", [CSH, NCH, SCH + 1, P], mf,
                         kind="ExternalInput").ap()
    am = nc.dram_tensor("am", [S, S], mf, kind="ExternalInput").ap()
    ob = nc.dram_tensor("ob", [1, FREE], mf, kind="ExternalInput").ap()
    out = nc.dram_tensor("out", [B, CSH, P], f32, kind="ExternalOutput").ap()

    with tile.TileContext(nc) as tc, ExitStack() as ctx:
        const = ctx.enter_context(tc.tile_pool(name="const", bufs=1))
        xpool = ctx.enter_context(tc.tile_pool(name="xp", bufs=1))
        spool = ctx.enter_context(tc.tile_pool(name="sp", bufs=1))
        wpool = ctx.enter_context(tc.tile_pool(name="wp", bufs=6))
        opool = ctx.enter_context(tc.tile_pool(name="op", bufs=4))
        pseas = ctx.enter_context(tc.tile_pool(name="pps", bufs=3, space="PSUM"))
        pmain = ctx.enter_context(tc.tile_pool(name="ppm", bufs=4, space="PSUM"))

        # Banded avg-pool operator blocks A[j-chunk, i-chunk] (only |i-j|<=1)
        a_sb = {}
        for i in range(NCH):
            for j in range(max(0, i - 1), min(NCH, i + 2)):
                t = const.tile([SCH, SCH], mf, tag=f"a{j}_{i}", name=f"a{j}_{i}")
                nc.sync.dma_start(t[:], am[j * SCH:(j + 1) * SCH,
                                            i * SCH:(i + 1) * SCH])
                a_sb[(j, i)] = t

        # Load x: 6 chunk tiles of [120, (c,b)=2624]
        xs = []
        for j in range(NCH):
            t = xpool.tile([SCH, FREE], mf, tag=f"x{j}", name=f"x{j}")
            nc.sync.dma_start(
                t[:], xt[j * SCH:(j + 1) * SCH].rearrange("p c b -> p (c b)"))
            xs.append(t)
        # season tiles have a 121st row held at 1.0: it multiplies the bias row
        # (chunk 5) / zero rows (other chunks) of the padded wd weight tiles
        seas = [spool.tile([SCH + 1, FREE], mf, tag=f"s{j}", name=f"s{j}")
                for j in range(NCH)]
        for j in range(NCH):
            nc.sync.dma_start(seas[j][SCH:SCH + 1, :], ob[:])

        n_nt = (FREE + NT - 1) // NT
        for nt in range(n_nt):
            c0 = nt * NT
            w = min(NT, FREE - c0)
            # season[i-chunk, cols] = sum_j A[j,i].T @ x[j-chunk, cols]
            for i in range(NCH):
                ps = pseas.tile([SCH, NT], f32, tag="ps")
                js = [j for j in (i - 1, i, i + 1) if 0 <= j < NCH]
                for idx, j in enumerate(js):
                    nc.tensor.matmul(ps[:, :w], a_sb[(j, i)][:],
                                     xs[j][:, c0:c0 + w],
                                     start=(idx == 0), stop=(idx == len(js) - 1))
                nc.vector.tensor_copy(seas[i][:SCH, c0:c0 + w], ps[:, :w])

            # channels whose (c,b) columns are fully covered by tiles <= nt
            ch_lo = c0 // B
            ch_hi = min(CSH, (c0 + w) // B)
            for c in range(ch_lo, ch_hi):
                wr = wpool.tile([SCH, NCH, P], mf, tag="wt")
                nc.sync.dma_start(
                    wr[:], wrt[c].rearrange("(j p) n -> p j n", p=SCH))
                wd = wpool.tile([SCH + 1, NCH, P], mf, tag="wt")
                nc.sync.dma_start(
                    wd[:], wdt[c].rearrange("j p n -> p j n"))

                pm = pmain.tile([B, P], f32, tag="pm")
                cb = slice(c * B, (c + 1) * B)
                for j in range(NCH):
                    nc.tensor.matmul(pm[:], xs[j][:, cb], wr[:, j, :],
                                     start=(j == 0), stop=False)
                for j in range(NCH):
                    nc.tensor.matmul(pm[:], seas[j][:, cb], wd[:, j, :],
                                     start=False, stop=(j == NCH - 1))

                ot = opool.tile([B, P], f32, tag="o")
                nc.vector.tensor_copy(ot[:], pm[:])
                nc.sync.dma_start(out[:, c, :], ot[:])

    nc.compile()
    return nc


def _host_prep(x, W_res, b_res, W_trend, b_trend):
    """Slice + transpose per-core inputs. All arrays float32."""
    amat = np.zeros((S, S), np.float32)
    idx = np.abs(np.arange(S)[:, None] - np.arange(S)[None, :]) <= PAD
    amat[idx] = np.float32(1.0 / WIN)

    w_diff = W_trend - W_res
    b_sum = b_res + b_trend

    in_maps = []
    for k in range(N_CORES):
        lo = k * CSH
        hi = min(C, lo + CSH)
        w = max(0, hi - lo)
        XT = np.zeros((S, CSH, B), np.float32)
        WRT = np.zeros((CSH, S, P), np.float32)
        WDT = np.zeros((CSH, NCH, SCH + 1, P), np.float32)
        if w > 0:
            XT[:, :w, :] = x[:, :, lo:hi].transpose(1, 2, 0)
            WRT[:w] = W_res[lo:hi].transpose(0, 2, 1)
            WDT[:w, :, :SCH, :] = (
                w_diff[lo:hi].transpose(0, 2, 1).reshape(w, NCH, SCH, P))
            WDT[:w, NCH - 1, SCH, :] = b_sum[lo:hi]
        in_maps.append({"xt": XT, "wrt": WRT, "wdt": WDT, "am": amat,
                        "ob": np.ones((1, FREE), np.float32)})
    return in_maps


def kernel(x, W_res, b_res, W_trend, b_trend):
    from concourse.bass_utils import run_bass_kernel_spmd

    if "nc" not in _cached:
        _cached["nc"] = _build_program()
    nc = _cached["nc"]

    in_maps = _host_prep(x, W_res, b_res, W_trend, b_trend)
    res = run_bass_kernel_spmd(nc, in_maps, core_ids=list(range(N_CORES)),
                               trace=False)
    _cached["last_results"] = res

    out = np.empty((B, C, P), np.float32)
    for k in range(N_CORES):
        lo = k * CSH
        hi = min(C, lo + CSH)
        if hi > lo:
            out[:, lo:hi, :] = res.results[k]["out"][:, :hi - lo, :]
    return out


# revision 14
# speedup vs baseline: 1.2235x; 1.2235x over previous
"""DLinear forward kernel for Trainium2, sharded over 8 NeuronCores.

Model (per channel c):
    season = AvgPool1d(x_c, k=25, pad=12, count_include_pad=True)
    out_c  = (x_c - season) @ W_res[c].T + season @ W_trend[c].T + b_res[c] + b_trend[c]
           = x_c @ W_res[c].T + season @ (W_trend[c]-W_res[c]).T + bias[c]

Sharding: channels split across cores (zero communication). Per core:
  - x slice pre-transposed to [S, Csh, B] so SBUF tiles are [s-chunk, (c,b)]
  - weights pre-transposed to [Csh, S, P] so matmul rhs tiles DMA contiguously
  - season computed on TensorE as a banded-matrix matmul (A is the avg-pool
    operator, symmetric, built host-side)
  - bias injected into PSUM via a K=1 matmul with a ones row
"""

import sys

sys.path.insert(0, "/opt/trn_rl_repo")

from contextlib import ExitStack

import numpy as np

B, S, C, P = 64, 720, 321, 336
WIN, PAD = 25, 12
N_CORES = 8
CSH = 41                      # padded channels per core (8*41 = 328 >= 321)
SCH, NCH = 120, 6             # S split into 6 chunks of 120 partitions
FREE = CSH * B                # 2624 columns in the (c, b) free dim
NT = 512                      # pooling matmul moving-dim tile
F32R = True                   # float32r matmuls (4-byte, full-rate at N>=256)

_cached = {}


def _build_program():
    import concourse.bacc as bacc
    import concourse.mybir as mybir
    import concourse.tile as tile

    f32 = mybir.dt.float32
    f32r = mybir.dt.float32r

    mf = f32r if F32R else f32  # dtype for all matmul operands

    nc = bacc.Bacc("TRN2", target_bir_lowering=False, debug=False,
                   num_devices=N_CORES)
    xt = nc.dram_tensor("xt", [S, CSH, B], mf, kind="ExternalInput").ap()
    # weights stored partition-major: [c, p, j, n] so each SBUF partition row
    # is one contiguous 8KB DMA descriptor
    wrt = nc.dram_tensor("wrt", [CSH, SCH, NCH, P], mf,
                         kind="ExternalInput").ap()
    # wd has a 121st partition row; its chunk-5 block carries the bias
    wdt = nc.dram_tensor("wdt", [CSH, SCH + 1, NCH, P], mf,
                         kind="ExternalInput").ap()
    am = nc.dram_tensor("am", [S, S], mf, kind="ExternalInput").ap()
    ob = nc.dram_tensor("ob", [1, FREE], mf, kind="ExternalInput").ap()
    out = nc.dram_tensor("out", [B, CSH, P], f32, kind="ExternalOutput").ap()

    with tile.TileContext(nc) as tc, ExitStack() as ctx:
        const = ctx.enter_context(tc.tile_pool(name="const", bufs=1))
        xpool = ctx.enter_context(tc.tile_pool(name="xp", bufs=1))
        spool = ctx.enter_context(tc.tile_pool(name="sp", bufs=1))
        wpool = ctx.enter_context(tc.tile_pool(name="wp", bufs=6))
        opool = ctx.enter_context(tc.tile_pool(name="op", bufs=4))
        pseas = ctx.enter_context(tc.tile_pool(name="pps", bufs=3, space="PSUM"))
        pmain = ctx.enter_context(tc.tile_pool(name="ppm", bufs=4, space="PSUM"))

        # Banded avg-pool operator blocks A[j-chunk, i-chunk] (only |i-j|<=1)
        a_sb = {}
        for i in range(NCH):
            for j in range(max(0, i - 1), min(NCH, i + 2)):
                t = const.tile([SCH, SCH], mf, tag=f"a{j}_{i}", name=f"a{j}_{i}")
                nc.gpsimd.dma_start(t[:], am[j * SCH:(j + 1) * SCH,
                                              i * SCH:(i + 1) * SCH])
                a_sb[(j, i)] = t

        # Load x: 6 chunk tiles of [120, (c,b)=2624]
        xs = []
        for j in range(NCH):
            t = xpool.tile([SCH, FREE], mf, tag=f"x{j}", name=f"x{j}")
            eng = nc.sync if j % 2 == 0 else nc.scalar
            eng.dma_start(
                t[:], xt[j * SCH:(j + 1) * SCH].rearrange("p c b -> p (c b)"))
            xs.append(t)
        # season tiles have a 121st row held at 1.0: it multiplies the bias row
        # (chunk 5) / zero rows (other chunks) of the padded wd weight tiles
        seas = [spool.tile([SCH + 1, FREE], mf, tag=f"s{j}", name=f"s{j}")
                for j in range(NCH)]
        for j in range(NCH):
            nc.gpsimd.dma_start(seas[j][SCH:SCH + 1, :], ob[:])

        n_nt = (FREE + NT - 1) // NT
        for nt in range(n_nt):
            c0 = nt * NT
            w = min(NT, FREE - c0)
            # season[i-chunk, cols] = sum_j A[j,i].T @ x[j-chunk, cols]
            for i in range(NCH):
                ps = pseas.tile([SCH, NT], f32, tag="ps")
                js = [j for j in (i - 1, i, i + 1) if 0 <= j < NCH]
                for idx, j in enumerate(js):
                    nc.tensor.matmul(ps[:, :w], a_sb[(j, i)][:],
                                     xs[j][:, c0:c0 + w],
                                     start=(idx == 0), stop=(idx == len(js) - 1))
                nc.vector.tensor_copy(seas[i][:SCH, c0:c0 + w], ps[:, :w])

            # channels whose (c,b) columns are fully covered by tiles <= nt
            ch_lo = c0 // B
            ch_hi = min(CSH, (c0 + w) // B)
            for c in range(ch_lo, ch_hi):
                wr = wpool.tile([SCH, NCH, P], mf, tag="wt")
                nc.sync.dma_start(wr[:], wrt[c])
                wd = wpool.tile([SCH + 1, NCH, P], mf, tag="wt")
                nc.scalar.dma_start(wd[:], wdt[c])

                pm = pmain.tile([B, P], f32, tag="pm")
                cb = slice(c * B, (c + 1) * B)
                for j in range(NCH):
                    nc.tensor.matmul(pm[:], xs[j][:, cb], wr[:, j, :],
                                     start=(j == 0), stop=False)
                for j in range(NCH):
                    nc.tensor.matmul(pm[:], seas[j][:, cb], wd[:, j, :],
                                     start=False, stop=(j == NCH - 1))

                ot = opool.tile([B, P], f32, tag="o")
                nc.vector.tensor_copy(ot[:], pm[:])
                nc.gpsimd.dma_start(out[:, c, :], ot[:])

    nc.compile()
    return nc


def _host_prep(x, W_res, b_res, W_trend, b_trend):
    """Slice + transpose per-core inputs. All arrays float32."""
    amat = np.zeros((S, S), np.float32)
    idx = np.abs(np.arange(S)[:, None] - np.arange(S)[None, :]) <= PAD
    amat[idx] = np.float32(1.0 / WIN)

    w_diff = W_trend - W_res
    b_sum = b_res + b_trend

    in_maps = []
    for k in range(N_CORES):
        lo = k * CSH
        hi = min(C, lo + CSH)
        w = max(0, hi - lo)
        XT = np.zeros((S, CSH, B), np.float32)
        WRT = np.zeros((CSH, SCH, NCH, P), np.float32)
        WDT = np.zeros((CSH, SCH + 1, NCH, P), np.float32)
        if w > 0:
            XT[:, :w, :] = x[:, :, lo:hi].transpose(1, 2, 0)
            # [c, P, S] -> [c, S, P] -> [c, j, p, n] -> [c, p, j, n]
            WRT[:w] = (W_res[lo:hi].transpose(0, 2, 1)
                       .reshape(w, NCH, SCH, P).transpose(0, 2, 1, 3))
            WDT[:w, :SCH] = (w_diff[lo:hi].transpose(0, 2, 1)
                             .reshape(w, NCH, SCH, P).transpose(0, 2, 1, 3))
            WDT[:w, SCH, NCH - 1, :] = b_sum[lo:hi]
        in_maps.append({"xt": XT, "wrt": WRT, "wdt": WDT, "am": amat,
                        "ob": np.ones((1, FREE), np.float32)})
    return in_maps


def kernel(x, W_res, b_res, W_trend, b_trend):
    from concourse.bass_utils import run_bass_kernel_spmd

    if "nc" not in _cached:
        _cached["nc"] = _build_program()
    nc = _cached["nc"]

    in_maps = _host_prep(x, W_res, b_res, W_trend, b_trend)
    res = run_bass_kernel_spmd(nc, in_maps, core_ids=list(range(N_CORES)),
                               trace=False)
    _cached["last_results"] = res

    out = np.empty((B, C, P), np.float32)
    for k in range(N_CORES):
        lo = k * CSH
        hi = min(C, lo + CSH)
        if hi > lo:
            out[:, lo:hi, :] = res.results[k]["out"][:, :hi - lo, :]
    return out
